# revision 37
# baseline (speedup 1.0000x reference)
"""Trainium2 Bass kernel for nn_AblatedModel_40802189312754 (2-layer GNN + scoring).

V3. Entities row-sharded 8 ways (6250/core, padded to 6400). Both SpMM layers
run over one uniform flat slot space: chunks of 128 edges binned by
(source-vid-half, dest-window, dest-bin), gather groups of 1024 slots.
Layer 1 streams host-pre-gathered edge features (2 groups per DMA); layer 2
DMA-gathers rows from the AllGathered bf16 h1 table (single collective,
vid halves for int16 indices). Each dest window accumulates in PSUM twice
(half-A pass copy, half-B pass add into SBUF). Scoring fp32 as baseline:
fet = E^T + h2^T local, logits column-sharded [B, 6400/core].
"""
import sys
sys.path.insert(0, '/opt/trn_rl_repo')

import numpy as np
import ml_dtypes

import concourse.bacc as bacc
import concourse.tile as tile
import concourse.mybir as mybir
from concourse.bass_utils import run_bass_kernel_spmd

BF16 = ml_dtypes.bfloat16

N_ENT = 50000
D = 128
B = 1024
NC = 8
SH = 6250            # real entities per shard
NSH = 6400           # padded shard size
NV = NC * NSH        # virtual table rows (51200)
HALF = 32768         # vid half boundary (int16 gather indices)
BN_EPS = 1e-5
SEGW = 32            # dest bin width (indicator cols per chunk)
GIDX = 1024          # gather group size (slots)
LPAIR = 2            # layer-1 groups per stream DMA
NWIN = 13
WINDOWS = [(w, min(512, NSH - w)) for w in range(0, NSH, 512)]


def _plan(rows, cols, vals):
    """Uniform cross-core plan over the flat slot space."""
    per_core = []
    for k in range(NC):
        m = (rows >= k * SH) & (rows < (k + 1) * SH)
        r = (rows[m] - k * SH).astype(np.int64)
        c = cols[m].astype(np.int64)
        v = vals[m].astype(np.float32)
        oc = c // SH
        vid = oc * NSH + (c - oc * SH)
        half = (vid >= HALF).astype(np.int64)
        tid = vid - half * HALF
        wi = np.minimum(r // 512, NWIN - 1)
        bin_id = (r - wi * 512) // SEGW
        nbin_w = [(wsz + SEGW - 1) // SEGW for (w0, wsz) in WINDOWS]
        bin_base = np.cumsum([0] + nbin_w)
        cell = half * bin_base[-1] + bin_base[wi] + bin_id
        o = np.lexsort((tid, cell))
        per_core.append((cell[o], r[o], tid[o], c[o], v[o]))

    nbin_w = [(wsz + SEGW - 1) // SEGW for (w0, wsz) in WINDOWS]
    ncell_h = sum(nbin_w)
    ncell = 2 * ncell_h
    cnt = np.zeros((NC, ncell), np.int64)
    for k in range(NC):
        np.add.at(cnt[k], per_core[k][0], 1)
    nch_cell = -(-cnt.max(axis=0) // 128)

    bin_cum = [0] + list(np.cumsum(nbin_w))
    chunks = []          # (half, wi, seg0_in_window)
    cell_ch0 = np.zeros(ncell + 1, np.int64)
    for cid in range(ncell):
        half = cid // ncell_h
        rel = cid - half * ncell_h
        wi = int(np.searchsorted(np.cumsum(nbin_w), rel, side='right'))
        seg0 = (rel - bin_cum[wi]) * SEGW
        cell_ch0[cid] = len(chunks)
        for _ in range(int(nch_cell[cid])):
            chunks.append((half, wi, seg0))
    cell_ch0[ncell] = len(chunks)
    nch = len(chunks)
    ncha = int(cell_ch0[ncell_h])
    ALIGN = GIDX * LPAIR
    slot_a = -(-(ncha * 128) // ALIGN) * ALIGN
    nslot = slot_a + -(-((nch - ncha) * 128) // ALIGN) * ALIGN

    def ch_slot0(ci):
        return ci * 128 if ci < ncha else slot_a + (ci - ncha) * 128

    struct = {'chunks': chunks, 'nslot': nslot, 'slot_a': slot_a,
              'ncha': ncha, 'ngroups_a': slot_a // GIDX,
              'ngroups': nslot // GIDX, 'nch': nch}

    ngroups = struct['ngroups']
    cores = []
    for k in range(NC):
        cell, r, tid, c, v = per_core[k]
        lo = np.searchsorted(cell, np.arange(ncell))
        hi = np.searchsorted(cell, np.arange(ncell) + 1)
        src = np.full(nslot, -1, np.int64)
        tix = np.zeros(nslot, np.int64)
        ind = np.zeros((nch, 128, SEGW), np.float32)
        for cid in range(ncell):
            a, e = int(lo[cid]), int(hi[cid])
            n = e - a
            if n == 0:
                continue
            ch0 = int(cell_ch0[cid])
            s0 = ch_slot0(ch0)
            src[s0:s0 + n] = c[a:e]
            tix[s0:s0 + n] = tid[a:e]
            ci = ch0 + np.arange(n) // 128
            si = np.arange(n) % 128
            seg = r[a:e] % SEGW
            ind[ci, si, seg] = v[a:e]
        ids = tix.reshape(ngroups, GIDX // 16, 16)
        idx = np.tile(np.ascontiguousarray(
            ids.transpose(2, 0, 1).reshape(16, ngroups * (GIDX // 16))),
            (8, 1)).astype(np.int16)
        cores.append({
            'ind': np.ascontiguousarray(
                ind.transpose(1, 0, 2).reshape(128, nch * SEGW)).astype(BF16),
            'idx': np.ascontiguousarray(idx),
            'src': src,
        })
    return struct, cores


def _build_nc(struct):
    nch = struct['nch']
    ng = struct['ngroups']
    nga = struct['ngroups_a']
    ncha = struct['ncha']
    slot_a = struct['slot_a']
    chunks = struct['chunks']
    GI16 = GIDX // 16

    def ch_slot0(ci):
        return ci * 128 if ci < ncha else slot_a + (ci - ncha) * 128

    nc = bacc.Bacc("TRN2", target_bir_lowering=False, debug=False,
                   enable_asserts=True, num_devices=NC, num_swdge_queues=4,
                   dynamic_dma_scratch_size=32768)
    f32, bf, i16 = mybir.dt.float32, mybir.dt.bfloat16, mybir.dt.int16
    AF = mybir.ActivationFunctionType

    g1_d = nc.dram_tensor("g1", [ng // LPAIR, 128, LPAIR * GIDX], bf,
                          kind="ExternalInput")
    ind_d = nc.dram_tensor("ind", [128, nch * SEGW], bf, kind="ExternalInput")
    idx_d = nc.dram_tensor("idx", [128, ng * GI16], i16, kind="ExternalInput")
    w1_d = nc.dram_tensor("w1", [D, D], f32, kind="ExternalInput")
    w2_d = nc.dram_tensor("w2", [D, D], f32, kind="ExternalInput")
    w_d = nc.dram_tensor("w", [D, D], f32, kind="ExternalInput")
    bn_d = nc.dram_tensor("bn", [D, 8], f32, kind="ExternalInput")
    et_d = nc.dram_tensor("et", [128, NSH], f32, kind="ExternalInput")
    ebh_d = nc.dram_tensor("ebh", [128, B], f32, kind="ExternalInput")
    rgt_d = nc.dram_tensor("rgt", [128, B], f32, kind="ExternalInput")
    bidx_d = nc.dram_tensor("bidx", [128, 64], i16, kind="ExternalInput")
    ident_d = nc.dram_tensor("ident", [D, D], f32, kind="ExternalInput")
    out_d = nc.dram_tensor("out", [8, 13, 128, 512], f32, kind="ExternalOutput")

    runs = []        # (half, wi, ch_lo, ch_hi)
    for ci, (half, wi, seg0) in enumerate(chunks):
        if runs and runs[-1][0] == half and runs[-1][1] == wi:
            runs[-1][3] = ci + 1
        else:
            runs.append([half, wi, ci, ci + 1])
    # xform_win is emitted per (half=1, wi) run; every window must have both
    # passes so s_t is initialized (pass A) and transformed (pass B)
    assert {(h, w) for h, w, _, _ in runs} == {(h, w) for h in (0, 1)
                                              for w in range(NWIN)}

    with tile.TileContext(nc) as tc:
        with tc.tile_pool(name="const", bufs=1) as cp, \
             tc.tile_pool(name="gp", bufs=8) as gp, \
             tc.tile_pool(name="indp", bufs=3) as indp, \
             tc.tile_pool(name="etp", bufs=2) as etp, \
             tc.tile_pool(name="sp", bufs=1) as spool, \
             tc.tile_pool(name="hp", bufs=1) as hp, \
             tc.tile_pool(name="hep", bufs=8) as hep, \
             tc.tile_pool(name="bp", bufs=1) as bp, \
             tc.tile_pool(name="op", bufs=3) as op, \
             tc.tile_pool(name="pch", bufs=2, space="PSUM") as pch, \
             tc.tile_pool(name="px", bufs=2, space="PSUM") as px, \
             tc.tile_pool(name="psc", bufs=4, space="PSUM") as psc, \
             tc.tile_pool(name="dram", bufs=2, space="DRAM") as dp:

            w1_t = cp.tile([D, D], f32); nc.sync.dma_start(w1_t[:], w1_d[:])
            w2_t = cp.tile([D, D], f32); nc.sync.dma_start(w2_t[:], w2_d[:])
            w_t = cp.tile([D, D], f32); nc.sync.dma_start(w_t[:], w_d[:])
            bn_t = cp.tile([D, 8], f32); nc.sync.dma_start(bn_t[:], bn_d[:])
            id_t = cp.tile([D, D], f32); nc.sync.dma_start(id_t[:], ident_d[:])
            ebh_t = cp.tile([128, B], f32); nc.sync.dma_start(ebh_t[:], ebh_d[:])
            rgt_t = cp.tile([128, B], f32); nc.sync.dma_start(rgt_t[:], rgt_d[:])
            bidx_t = cp.tile([128, 64], i16); nc.sync.dma_start(bidx_t[:], bidx_d[:])
            zl_t = cp.tile([1, 128], bf); nc.vector.memset(zl_t[:], 0.0)
            zr_t = cp.tile([1, 512], bf); nc.vector.memset(zr_t[:], 0.0)
            idb_t = cp.tile([D, D], bf)
            nc.vector.tensor_copy(idb_t[:], id_t[:])

            idx_all = cp.tile([128, ng * GI16], i16)
            nc.sync.dma_start(idx_all[:], idx_d[:])

            s_t = spool.tile([128, NSH], f32, tag="s")
            h1t_t = hp.tile([128, NSH], bf, tag="h1t")
            h2t_t = hp.tile([128, NSH], f32, tag="h2t")
            h2tb_t = hp.tile([128, NSH], bf, tag="h2tb")

            hsh = dp.tile([NSH, D], bf, tag="hsh")
            h2sh = dp.tile([NSH, D], bf, tag="h2sh")
            hfull = dp.tile([NV, D], bf, tag="hfull", addr_space="Shared")

            def spmm(layer):
                qn = 0
                gtiles = {}

                def get_g(key):
                    nonlocal qn
                    if key in gtiles:
                        return gtiles[key]
                    if layer == 0:
                        g_t = gp.tile([128, LPAIR * GIDX], bf, tag="g",
                                      name=f"g_l0_{key}")
                        nc.sync.dma_start(g_t[:], g1_d[key])
                    else:
                        g_t = gp.tile([128, GIDX], bf, tag="g",
                                      name=f"g_l1_{key}")
                        src = (hfull[0:HALF] if key < nga
                               else hfull[HALF:NV])
                        nc.gpsimd.dma_gather(
                            g_t[:].rearrange("p (c e) -> p c e", e=D),
                            src, idx_all[:, key * GI16:(key + 1) * GI16],
                            GIDX, GIDX, D, queue_num=qn % 4)
                        qn += 1
                    gtiles[key] = g_t
                    return g_t

                for half, wi, lo, hi in runs:
                    w0, wsz = WINDOWS[wi]
                    ind_t = indp.tile([128, (hi - lo) * SEGW], bf, tag="ind",
                                      name=f"ind_l{layer}_{half}_{wi}")
                    nc.sync.dma_start(ind_t[:],
                                      ind_d[:, lo * SEGW:hi * SEGW])
                    ps = pch.tile([128, 512], f32, tag="ps")
                    nc.tensor.matmul(ps[:], zl_t[:], zr_t[:],
                                     start=True, stop=False,
                                     skip_group_check=True)
                    for ci in range(lo, hi):
                        _, _, seg0 = chunks[ci]
                        s0 = ch_slot0(ci)
                        if layer == 0:
                            g_t = get_g(s0 // (LPAIR * GIDX))
                            cb = (s0 % (LPAIR * GIDX)) // 128
                        else:
                            g_t = get_g(s0 // GIDX)
                            cb = (s0 % GIDX) // 128
                        nc.tensor.matmul(
                            ps[:, seg0:seg0 + SEGW],
                            g_t[:, cb * 128:(cb + 1) * 128],
                            ind_t[:, (ci - lo) * SEGW:(ci - lo + 1) * SEGW],
                            start=False, stop=(ci == hi - 1),
                            skip_group_check=True)
                    if half == 0:
                        nc.vector.tensor_copy(s_t[:, w0:w0 + wsz], ps[:, :wsz])
                    else:
                        nc.vector.tensor_tensor(s_t[:, w0:w0 + wsz],
                                                s_t[:, w0:w0 + wsz],
                                                ps[:, :wsz],
                                                mybir.AluOpType.add)
                        # in layer 2 the PE idles under the gather stream, so
                        # emit the window's xform inline to shorten the tail
                        if layer == 1:
                            xform_win(1, wi)
                if layer == 0:
                    for wi in range(NWIN):
                        xform_win(0, wi)
                    nc.gpsimd.collective_compute(
                        "AllGather", mybir.AluOpType.bypass,
                        replica_groups=[list(range(NC))],
                        ins=[hsh[:].opt()], outs=[hfull[:].opt()])

            def store_rm(src_t, dst_dram, w0, wsz, layer):
                # PE transpose + DVE copy + plain store (no xbar: it would
                # serialize against the SWDGE gather stream)
                for t in range(w0 // 128, (w0 + wsz) // 128):
                    pt = psc.tile([128, 128], bf, tag="sc",
                                  name=f"pt_l{layer}_{t}")
                    nc.tensor.transpose(pt[:], src_t[:, t * 128:(t + 1) * 128],
                                        idb_t[:])
                    hent = hep.tile([128, 128], bf, tag="hent")
                    nc.vector.tensor_copy(hent[:], pt[:])
                    nc.scalar.dma_start(dst_dram[t * 128:(t + 1) * 128, :],
                                        hent[:])

            def xform_win(layer, wi):
                wmat = w1_t if layer == 0 else w2_t
                bcol = bn_t[:, 0:1] if layer == 0 else bn_t[:, 1:2]
                w0, wsz = WINDOWS[wi]
                xp = px.tile([128, 512], f32, tag="xp")
                nc.tensor.matmul(xp[:, :wsz], wmat[:], s_t[:, w0:w0 + wsz],
                                 start=True, stop=True)
                if layer == 0:
                    nc.scalar.activation(h1t_t[:, w0:w0 + wsz], xp[:, :wsz],
                                         AF.Relu, bias=bcol, scale=1.0)
                    store_rm(h1t_t, hsh, w0, wsz, 0)
                else:
                    nc.scalar.activation(h2t_t[:, w0:w0 + wsz], xp[:, :wsz],
                                         AF.Relu, bias=bcol, scale=1.0)
                    nc.vector.tensor_copy(h2tb_t[:, w0:w0 + wsz],
                                          h2t_t[:, w0:w0 + wsz])
                    if wi == NWIN - 1:
                        nc.vector.memset(h2tb_t[:, NSH - 1:NSH], 0.0)
                    store_rm(h2tb_t, h2sh, w0, wsz, 1)
                    # fold E^T into h2t now (h2tb holds the pure copy);
                    # scoring then starts straight after the AllReduce
                    etw = etp.tile([128, 512], f32, tag="etw")
                    nc.sync.dma_start(etw[:, :wsz], et_d[:, w0:w0 + wsz])
                    nc.vector.tensor_tensor(h2t_t[:, w0:w0 + wsz],
                                            h2t_t[:, w0:w0 + wsz],
                                            etw[:, :wsz],
                                            mybir.AluOpType.add)

            # ---- layer 1 ----
            spmm(0)

            # ---- layer 2 ----
            spmm(1)

            # batch tail
            tlo = bp.tile([128, B], bf)
            nc.gpsimd.dma_gather(
                tlo[:].rearrange("p (c e) -> p c e", e=D), h2sh[:],
                bidx_t[:, 0:64], B, B, D, queue_num=0)
            xpart = bp.tile([128, B], f32)
            nc.vector.tensor_copy(xpart[:], tlo[:])
            xin_dram = dp.tile([128, B], f32, tag="xin")
            xout_dram = dp.tile([128, B], f32, tag="xout", addr_space="Shared")
            nc.sync.dma_start(xin_dram[:], xpart[:])
            nc.gpsimd.collective_compute(
                "AllReduce", mybir.AluOpType.add,
                replica_groups=[list(range(NC))],
                ins=[xin_dram[:].opt()], outs=[xout_dram[:].opt()])
            xraw = bp.tile([128, B], f32)
            nc.sync.dma_start(xraw[:], xout_dram[:])
            nc.vector.tensor_tensor(xraw[:], xraw[:], ebh_t[:],
                                    mybir.AluOpType.add)
            xtb = bp.tile([128, B], f32)
            for j in range(8):
                tp = px.tile([128, 128], f32, tag="xp")
                nc.tensor.transpose(tp[:], xraw[:, j * 128:(j + 1) * 128],
                                    id_t[:])
                nc.vector.tensor_scalar(
                    xtb[:, j * 128:(j + 1) * 128], tp[:],
                    bn_t[:, 2:3], bn_t[:, 3:4],
                    mybir.AluOpType.mult, mybir.AluOpType.add)
            vmt = bp.tile([128, B], f32)
            for hb in range(2):
                sl = slice(hb * 512, hb * 512 + 512)
                wmp = px.tile([128, 512], f32, tag="xp")
                nc.tensor.matmul(wmp[:], w_t[:], rgt_t[:, sl],
                                 start=True, stop=True)
                nc.vector.tensor_tensor(vmt[:, sl], xtb[:, sl], wmp[:],
                                        mybir.AluOpType.mult)
            nc.vector.tensor_scalar(vmt[:], vmt[:], bn_t[:, 4:5], bn_t[:, 5:6],
                                    mybir.AluOpType.mult, mybir.AluOpType.add)

            # scoring fp32: h2t already holds fet = E^T + h2^T
            for bt in range(8):
                for wi, (w0, wsz) in enumerate(WINDOWS):
                    sc = psc.tile([128, 512], f32, tag="sc")
                    nc.tensor.matmul(sc[:, :wsz],
                                     vmt[:, bt * 128:(bt + 1) * 128],
                                     h2t_t[:, w0:w0 + wsz],
                                     start=True, stop=True)
                    ob = op.tile([128, 512], f32, tag="ob")
                    nc.scalar.activation(ob[:, :wsz], sc[:, :wsz], AF.Sigmoid)
                    nc.sync.dma_start(out_d[bt, wi, :, :wsz], ob[:, :wsz])
    nc.compile()
    return nc


def _host_prep(inputs):
    rows = np.asarray(inputs["adj_rows"]).astype(np.int64)
    cols = np.asarray(inputs["adj_cols"]).astype(np.int64)
    vals = np.asarray(inputs["adj_vals"], np.float32)
    E = np.asarray(inputs["E_emb"], np.float32)[np.asarray(inputs["init_ind"])]
    E_bf = E.astype(BF16)
    bh = np.asarray(inputs["batch_head"]).astype(np.int64)
    rel = np.asarray(inputs["batch_rel"]).astype(np.int64)
    R = np.asarray(inputs["R_emb"], np.float32)

    g0 = np.asarray(inputs["bn0_gamma"], np.float32) / np.sqrt(1.0 + BN_EPS)
    b0 = np.asarray(inputs["bn0_beta"], np.float32)
    g1v = np.asarray(inputs["bn1_gamma"], np.float32) / np.sqrt(1.0 + BN_EPS)
    b1v = np.asarray(inputs["bn1_beta"], np.float32)
    bn = np.ascontiguousarray(np.stack(
        [np.asarray(inputs["b1"], np.float32),
         np.asarray(inputs["b2"], np.float32),
         g0, b0, g1v, b1v,
         np.zeros(D, np.float32), np.zeros(D, np.float32)], axis=1))

    bh_owner = bh // SH
    bh_local = bh - bh_owner * SH

    def slot_layout(a):
        return np.ascontiguousarray(
            a.reshape(8, 128, D).transpose(1, 0, 2).reshape(128, 8 * D))

    ebh_l = slot_layout(E[bh])
    rgt = np.ascontiguousarray(R[rel].T.astype(np.float32))

    struct, cores = _plan(rows, cols, vals)
    ng = struct['ngroups']

    def wrap1024(ids):
        w = ids.reshape(64, 16).T
        return np.ascontiguousarray(np.tile(w, (8, 1)).astype(np.int16))

    in_maps = []
    for k in range(NC):
        pl = cores[k]
        src = pl['src']
        gmat = np.zeros((struct['nslot'], D), BF16)
        m = src >= 0
        gmat[m] = E_bf[src[m]]
        gw = LPAIR * GIDX
        g1 = np.ascontiguousarray(
            gmat.reshape(ng // LPAIR, gw // 128, 128, D)
            .transpose(0, 2, 1, 3).reshape(ng // LPAIR, 128, gw))
        et = np.zeros((128, NSH), np.float32)
        et[:, :SH] = E[k * SH:(k + 1) * SH].T
        in_maps.append({
            "g1": g1,
            "ind": pl['ind'],
            "idx": pl['idx'],
            "w1": np.asarray(inputs["W1"], np.float32),
            "w2": np.asarray(inputs["W2"], np.float32),
            "w": np.asarray(inputs["W"], np.float32),
            "bn": bn, "et": et, "ebh": ebh_l, "rgt": rgt,
            "bidx": wrap1024(np.where(bh_owner == k, bh_local,
                                      NSH - 1).astype(np.int64)),
            "ident": np.eye(D, dtype=np.float32),
        })
    return struct, in_maps


def _run(inputs, trace=False):
    struct, in_maps = _host_prep(inputs)
    nc = _build_nc(struct)
    res = run_bass_kernel_spmd(nc, in_maps, core_ids=list(range(NC)),
                               trace=trace)
    outs = []
    for k in range(NC):
        o = res.results[k]["out"]            # [8, 13, 128, 512]
        o = o.transpose(0, 2, 1, 3).reshape(B, NWIN * 512)[:, :NSH]
        outs.append(o[:, :SH])
    return np.concatenate(outs, axis=1).astype(np.float32), res


def kernel(**inputs):
    out, _ = _run(inputs, trace=False)
    return out
